# revision 26
# baseline (speedup 1.0000x reference)
"""AttentionBlock Trainium2 Bass kernel (v3: exp offload + mixed-dtype AV).

Full-input contract: kernel(**inputs) takes the complete tensors from
setup_inputs() and returns the full (4, 256, 64, 64) float32 output.

Sharding: 8 cores = 4 batches x 2 query-token halves (same as v1).

v1 was ACT-bound: 32 exp tiles x 1.04us = 33.2us per (qb, h) block while
PE/DVE idled.  v3 splits the exp stream and keeps the PE busy enough that
the HAM clock gate (PE at 1.2 GHz unless near-continuously busy) stays
warm:
  - exp: 10 chunk-pairs/block on ACT (exact Exp -> fp8e4 esc), 6 pairs on
    DVE (Schraudolph bit trick: int16(A*S+B) bitcast as bf16 ~= exp(S/8),
    ~3.3% max elementwise; the common e^-EXPC scale factor on both paths
    cancels in the softmax normalization).
  - AV: fp8 pairs use one DoubleRow matmul per 512 queries ([128,2,65+pad]
    weights, k-tile step %16==0 per the dual-fp8 ISA rule) contracting
    both key chunks at once - half the AV instructions; bf16 pairs keep
    the v1 path.  All accumulate into the same fp32 PSUM avp (ones-row Z).
  - scores: v1 bf16 row-group pairing (S0 from ksb/qsb in one 64-row
    half, S1 from the swapped copies in the other half, concurrent).
  - block 0 absorbs the K/Q t1+ projections (PSUM borrowed from the
    scores pool, ACT evac + swap DMAs) and the whole V projection (PE
    matmuls + DVE evacs), so the first exp starts right after the t0
    projections instead of behind V.
  - head order in a qb is [1, 3, 0, 2] (last norm takes the direct-write
    hp==0 path); qb0 output projection emitted mid-(qb1) borrowing the
    scores pool, so the drain is just the qb1 projection.
"""

import sys

sys.path.insert(0, "/opt/trn_rl_repo")

import numpy as np

# hardcoded problem geometry
B, C, H, W = 4, 256, 64, 64
NTOK = H * W            # 4096 keys per image
NLOC = NTOK // 2        # 2048 queries per core
QB = 1024               # query block (scores psum tile free size)
HEADS, D = 4, 64
GROUPS, CPG = 8, 32     # 8 groups x 32 channels
EPS = 1e-5
NCH = 32                # key chunks of 128
NP2 = 16                # chunk pairs per block

# Both exp paths compute exp(0.125*S - EXPC): the common e^-EXPC factor
# cancels in the softmax normalization and keeps exp(.) under the fp8e4
# max (448) for scores up to ~8.7 sigma.
EXPC = 2.0
LOG2E = 1.4426950408889634
EXPA = 0.125 * LOG2E * 128.0
EXPB = 127.0 * 128.0 - 5.6 - EXPC * LOG2E * 128.0

# fixed chunk-pair split: ACT pairs get exact Exp -> fp8 esc (DoubleRow
# AV, VT8); DVE pairs get the Schraudolph bf16 esc (v1-style bf16 AV,
# VTb).  GpSimd cannot read PSUM on real HW, so two engines it is.
DVE_PAIRS = (3, 5, 7, 9, 11, 13)
GPS_PAIRS = ()
BF16_PAIRS = tuple(sorted(DVE_PAIRS + GPS_PAIRS))
ACT_PAIRS = tuple(j2 for j2 in range(NP2) if j2 not in BF16_PAIRS)
VT8_SLOT = {j2: i for i, j2 in enumerate(ACT_PAIRS)}
VTB_SLOT = {}
for _i, _j2 in enumerate(BF16_PAIRS):
    VTB_SLOT[2 * _j2] = 2 * _i
    VTB_SLOT[2 * _j2 + 1] = 2 * _i + 1
N8 = len(ACT_PAIRS)                   # 8 fp8 pairs
NB = 2 * len(BF16_PAIRS)              # 16 bf16 chunks
VP = 80                               # fp8 V per-head stride (step%16==0)

_cached = {}


def _build_nc(num_devices=8):
    import concourse.mybir as mybir
    import concourse.tile as tile
    from concourse import bacc
    from concourse.bass import ds, ts

    fp32 = mybir.dt.float32
    bf16 = mybir.dt.bfloat16
    fp8 = mybir.dt.float8e4
    i16 = mybir.dt.int16
    AF = mybir.ActivationFunctionType
    OP = mybir.AluOpType
    AX = mybir.AxisListType
    PM = mybir.MatmulPerfMode

    nc = bacc.Bacc("TRN2", target_bir_lowering=False, debug=False,
                   num_devices=num_devices)

    xb = nc.dram_tensor("xb", [C, NTOK], fp32, kind="ExternalInput").ap()
    wqT = nc.dram_tensor("wqT", [C, C], fp32, kind="ExternalInput").ap()
    wkT = nc.dram_tensor("wkT", [C, C], fp32, kind="ExternalInput").ap()
    wvT = nc.dram_tensor("wvT", [C, C], fp32, kind="ExternalInput").ap()
    wpT = nc.dram_tensor("wpT", [C, C], fp32, kind="ExternalInput").ap()
    bqc = nc.dram_tensor("bqc", [C, 1], fp32, kind="ExternalInput").ap()
    bkc = nc.dram_tensor("bkc", [C, 1], fp32, kind="ExternalInput").ap()
    bpc2 = nc.dram_tensor("bpc2", [C, 1], fp32, kind="ExternalInput").ap()
    gnw = nc.dram_tensor("gnw", [C, 1], fp32, kind="ExternalInput").ap()
    gnb = nc.dram_tensor("gnb", [C, 1], fp32, kind="ExternalInput").ap()
    yo = nc.dram_tensor("y", [C, NLOC], fp32, kind="ExternalOutput").ap()

    from contextlib import ExitStack

    with tile.TileContext(nc) as tc, ExitStack() as ctx:
        pool = lambda name, bufs: ctx.enter_context(tc.tile_pool(name=name, bufs=bufs))
        consts = pool("consts", 1)
        otp = pool("ot", 1)
        xkp = pool("xk", 1)
        xb2p = pool("xb2", 1)

        # ---- x loads first (critical path) ----
        xh = {}
        for c2 in range(2):
            for hf in range(2):
                if hf == 0:
                    t = xkp.tile([128, NLOC], fp32, tag=f"xk{c2}",
                                 name=f"x{c2}h{hf}")
                else:
                    t = consts.tile([128, NLOC], fp32, tag=f"xt{c2}",
                                    name=f"x{c2}h{hf}")
                for q4 in range(4):
                    nc.sync.dma_start(
                        t[:, ds(q4 * 512, 512)],
                        xb[ts(c2, 128), ds(hf * NLOC + q4 * 512, 512)],
                    )
                xh[(c2, hf)] = t

        # ---- constants ----
        ones_col = consts.tile([1, 128], bf16, tag="ones_col")
        nc.gpsimd.memset(ones_col[:], 1.0)
        eps4 = consts.tile([4, 1], fp32, tag="eps4")
        nc.gpsimd.memset(eps4[:], EPS)
        ones64f = consts.tile([1, D], fp32, tag="ones64f")
        nc.gpsimd.memset(ones64f[:], 1.0)
        negc_col = consts.tile([128, 1], fp32, tag="negc")
        nc.gpsimd.memset(negc_col[:], -EXPC)
        mask4T = consts.tile([128, 4], fp32, tag="mask4T")
        nc.gpsimd.memset(mask4T[:], 0.0)
        for gl in range(4):
            nc.gpsimd.memset(
                mask4T[gl * CPG : (gl + 1) * CPG, gl : gl + 1],
                1.0 / (CPG * NTOK),
            )
        mask4B = consts.tile([4, 128], fp32, tag="mask4B")
        nc.gpsimd.memset(mask4B[:], 0.0)
        for gl in range(4):
            nc.sync.dma_start(
                mask4B[gl : gl + 1, gl * CPG : (gl + 1) * CPG],
                ones64f[0:1, 0:CPG],
            )

        bcols = {}
        for nm, src in (("q", bqc), ("k", bkc), ("gw", gnw), ("gb", gnb),
                        ("p2", bpc2)):
            for k2 in range(2):
                t = consts.tile([128, 1], fp32, tag=f"b{nm}{k2}")
                nc.sync.dma_start(t[:], src[ts(k2, 128), :])
                bcols[(nm, k2)] = t

        # weights to bf16 (DVE idle at startup); k/q first (gate first scores)
        wb = {}
        with tc.tile_pool(name="wload", bufs=4) as wldp:
            for nm, srcw in (("k", wkT), ("q", wqT), ("v", wvT), ("p", wpT)):
                for k2 in range(2):
                    t = wldp.tile([128, C], fp32, tag="wf",
                                  name=f"wf_{nm}{k2}")
                    nc.sync.dma_start(t[:], srcw[ts(k2, 128), :])
                    tb = consts.tile([128, C], bf16, tag=f"w{nm}b{k2}",
                                     name=f"w{nm}b{k2}")
                    nc.vector.tensor_copy(tb[:], t[:])
                    wb[(nm, k2)] = tb

        # V tables: fp8 pairs (DoubleRow, per-head stride VP so the k-tile
        # step HEADS*VP is %16==0 - dual-fp8 ISA rule) and bf16 chunks
        VT8 = consts.tile([128, N8, 2, HEADS, VP], fp8, tag="VT8")
        nc.gpsimd.memset(VT8[:, :, :, :, D : D + 1], 1.0)
        VTb = consts.tile([128, NB, HEADS, D + 1], bf16, tag="VTb")
        nc.gpsimd.memset(VTb[:, :, :, D : D + 1], 1.0)

        with tc.tile_pool(name="kq", bufs=1) as kqpool:
            ksb = [kqpool.tile([128, NTOK], bf16, tag=f"ksb{m}", name=f"ksb{m}")
                   for m in range(2)]
            qsb = [kqpool.tile([128, NLOC], bf16, tag=f"qsb{m}", name=f"qsb{m}")
                   for m in range(2)]
            ksw = [kqpool.tile([128, NTOK], bf16, tag=f"ksw{m}", name=f"ksw{m}")
                   for m in range(2)]
            qsw = [kqpool.tile([128, NLOC], bf16, tag=f"qsw{m}", name=f"qsw{m}")
                   for m in range(2)]
            with tc.tile_pool(name="xn", bufs=1) as xnpool:
                xn = [xnpool.tile([128, NTOK], bf16, tag=f"xn{c2}", name=f"xn{c2}")
                      for c2 in range(2)]

                # ---- group-norm ----
                with tc.tile_pool(name="stat", bufs=2) as statp, \
                     tc.tile_pool(name="gnps", bufs=2, space="PSUM") as gnps:
                    for c2 in range(2):
                        sacc = statp.tile([128, 4], fp32, tag="sacc")
                        scr = statp.tile([128, NLOC], bf16, tag="scr", bufs=1)
                        for hf in range(2):
                            nc.scalar.activation(
                                scr[:], xh[(c2, hf)][:], AF.Square,
                                accum_out=sacc[:, 2 + hf : 3 + hf],
                            )
                            nc.vector.tensor_reduce(
                                sacc[:, hf : hf + 1], xh[(c2, hf)][:],
                                axis=AX.X, op=OP.add,
                            )
                        me2 = statp.tile([128, 2], fp32, tag="me2")
                        nc.vector.tensor_add(
                            me2[:, 0:1], sacc[:, 0:1], sacc[:, 1:2]
                        )
                        nc.vector.tensor_add(
                            me2[:, 1:2], sacc[:, 2:3], sacc[:, 3:4]
                        )
                        gmp = gnps.tile([4, 2], fp32, tag="gmp")
                        nc.tensor.matmul(gmp[:], mask4T[:], me2[:])
                        gmsb = statp.tile([4, 2], fp32, tag="gmsb")
                        nc.vector.tensor_copy(gmsb[:], gmp[:])
                        gvar = statp.tile([4, 1], fp32, tag="gvar")
                        nc.vector.tensor_tensor(
                            gvar[:], gmsb[:, 0:1], gmsb[:, 0:1], op=OP.mult
                        )
                        nc.vector.tensor_tensor(
                            gvar[:], gmsb[:, 1:2], gvar[:], op=OP.subtract
                        )
                        gstd = statp.tile([4, 1], fp32, tag="gstd")
                        nc.scalar.activation(gstd[:], gvar[:], AF.Sqrt,
                                             bias=eps4[:])
                        grstd = statp.tile([4, 1], fp32, tag="grstd")
                        nc.vector.reciprocal(grstd[:], gstd[:])
                        rcolp = gnps.tile([128, 1], fp32, tag="rcolp")
                        nc.tensor.matmul(rcolp[:], mask4B[:], grstd[:])
                        mcolp = gnps.tile([128, 1], fp32, tag="mcolp")
                        nc.tensor.matmul(mcolp[:], mask4B[:], gmsb[:, 0:1])
                        acol = statp.tile([128, 1], fp32, tag="acol")
                        nc.vector.tensor_tensor(
                            acol[:], rcolp[:], bcols[("gw", c2)][:], op=OP.mult
                        )
                        bcol = statp.tile([128, 1], fp32, tag="bcol")
                        nc.vector.tensor_tensor(
                            bcol[:], mcolp[:], acol[:], op=OP.mult
                        )
                        nc.vector.tensor_tensor(
                            bcol[:], bcols[("gb", c2)][:], bcol[:], op=OP.subtract
                        )
                        # xn in quarters so K/Q t0 start after the first two
                        for hf in range(2):
                            for q2 in range(2):
                                nc.vector.tensor_scalar(
                                    xn[c2][:, ds(hf * NLOC + q2 * 1024, 1024)],
                                    xh[(c2, hf)][:, ds(q2 * 1024, 1024)],
                                    acol[:], bcol[:], op0=OP.mult, op1=OP.add,
                                )

                def kq_mm(pk, kind, m, t):
                    for half in range(2):
                        for k2 in range(2):
                            nc.tensor.matmul(
                                pk[:, ds(half * 512, 512)],
                                wb[(kind, k2)][:, ts(m, 128)],
                                xn[k2][:, ds(t * 1024 + half * 512, 512)],
                                start=(k2 == 0), stop=(k2 == 1),
                            )

                def kq_evac(pk, kind, m, t):
                    sb, sw = (ksb, ksw) if kind == "k" else (qsb, qsw)
                    nc.scalar.activation(
                        sb[m][:, ds(t * 1024, 1024)], pk[:],
                        AF.Identity, bias=bcols[(kind, m)][:],
                    )
                    nc.sync.dma_start(sw[m][0:64, ds(t * 1024, 1024)],
                                      sb[m][64:128, ds(t * 1024, 1024)])
                    nc.sync.dma_start(sw[m][64:128, ds(t * 1024, 1024)],
                                      sb[m][0:64, ds(t * 1024, 1024)])

                # K/Q t0 (both m halves): the only projections ahead of
                # block 0 (qb0 scores need q cols 0-1023 = t0 only)
                with tc.tile_pool(name="qkps", bufs=2, space="PSUM") as qkps:
                    for kind in ("k", "q"):
                        for m in range(2):
                            pk = qkps.tile([128, 1024], fp32, tag="pk",
                                           name=f"p{kind}{m}0")
                            kq_mm(pk, kind, m, 0)
                            kq_evac(pk, kind, m, 0)

                # remaining K/Q tiles ride block 0 (scores-pool PSUM)
                KQ_REST = [("k", 0, 1), ("k", 1, 1), ("k", 0, 2), ("k", 1, 2),
                           ("k", 0, 3), ("k", 1, 3), ("q", 0, 1), ("q", 1, 1)]

                # ---- attention ----
                outT = [otp.tile([128, NLOC], bf16, tag=f"outT{m}",
                                 name=f"outT{m}") for m in range(2)]
                HORDER = [1, 3, 0, 2]
                with tc.tile_pool(name="esc", bufs=6) as escp, \
                     tc.tile_pool(name="oa", bufs=2) as oap, \
                     tc.tile_pool(name="rzp", bufs=2) as rzp, \
                     tc.tile_pool(name="tmpn", bufs=2) as tmpp, \
                     tc.tile_pool(name="ys", bufs=2) as ysp, \
                     tc.tile_pool(name="scps", bufs=3, space="PSUM") as scps, \
                     tc.tile_pool(name="avps", bufs=1, space="PSUM") as avps:

                    def emit_av(avp, h, j2, esc, first, last):
                        if isinstance(esc, tuple):  # bf16 pair (e0, e1)
                            for c in range(2):
                                jv = VTB_SLOT[2 * j2 + c]
                                for t in range(2):
                                    nc.tensor.matmul(
                                        avp[:, ts(t, 512)],
                                        VTb[:, jv, h, :],
                                        esc[c][:, ts(t, 512)],
                                        start=(first and c == 0),
                                        stop=(last and c == 1),
                                        skip_group_check=True,
                                    )
                        else:  # fp8 DoubleRow: both chunks at once
                            s8 = VT8_SLOT[j2]
                            e2 = esc[:].rearrange("p (c n) -> p c n", c=2)
                            for t in range(2):
                                nc.tensor.matmul(
                                    avp[:, ts(t, 512)],
                                    VT8[:, s8, :, h, 0 : D + 1],
                                    e2[:, :, ds(t * 512, 512)],
                                    start=first, stop=last,
                                    perf_mode=PM.DoubleRow,
                                    skip_group_check=True,
                                )

                    def emit_norm(dn):
                        dth, dhp, dqb, doa, drzc = dn
                        dzbc = scps.tile([D, QB], fp32, tag="sc", name="zbc")
                        for t in range(2):
                            nc.tensor.matmul(
                                dzbc[:, ts(t, 512)], ones_col[0:1, 0:D],
                                drzc[0:1, ds(t * 512, 512)],
                            )
                        if dhp == 0:
                            nc.vector.tensor_tensor(
                                outT[dth][0:D, ds(dqb * QB, QB)], doa[0:D, :],
                                dzbc[:], op=OP.mult,
                            )
                        else:
                            tm = tmpp.tile([D, QB], bf16, tag="tm")
                            nc.vector.tensor_tensor(tm[:], doa[0:D, :],
                                                    dzbc[:], op=OP.mult)
                            nc.sync.dma_start(
                                outT[dth][64:128, ds(dqb * QB, QB)], tm[:]
                            )

                    deferred = None
                    carry = None
                    carry_meta = None
                    xb2 = None

                    for qb in range(2):
                        for hi, h in enumerate(HORDER):
                            th, hp = h // 2, h % 2
                            blk0 = (qb == 0 and hi == 0)
                            projblk = (qb == 1 and hi == 1)
                            b0 = hp * 64
                            b1 = 64 - b0
                            avp = avps.tile([D + 1, QB], fp32, tag="av",
                                            name=f"avp{qb}{h}")
                            pending = []
                            for j2 in range(NP2):
                                jj = 2 * j2
                                # paired scores: S0 from ksb/qsb in one
                                # 64-row group, S1 from the swapped copies
                                # in the other (concurrent row tiling)
                                S0 = scps.tile([128, QB], fp32, tag="sc",
                                               name="S0")
                                S1 = scps.tile([128, QB], fp32, tag="sc",
                                               name="S1")
                                for t in range(2):
                                    nc.tensor.matmul(
                                        S0[:, ts(t, 512)],
                                        ksb[th][b0 : b0 + 64, ts(jj, 128)],
                                        qsb[th][b0 : b0 + 64,
                                                ds(qb * QB + t * 512, 512)],
                                    )
                                    nc.tensor.matmul(
                                        S1[:, ts(t, 512)],
                                        ksw[th][b1 : b1 + 64, ts(jj + 1, 128)],
                                        qsw[th][b1 : b1 + 64,
                                                ds(qb * QB + t * 512, 512)],
                                    )
                                # exp
                                if j2 in BF16_PAIRS:
                                    eng = (nc.gpsimd if j2 in GPS_PAIRS
                                           else nc.vector)
                                    e0 = escp.tile([128, QB], bf16, tag="eb",
                                                   name="eb0", bufs=6)
                                    e1 = escp.tile([128, QB], bf16, tag="eb",
                                                   name="eb1", bufs=6)
                                    eng.tensor_scalar(
                                        e0[:].bitcast(i16), S0[:],
                                        EXPA, EXPB, op0=OP.mult, op1=OP.add,
                                    )
                                    eng.tensor_scalar(
                                        e1[:].bitcast(i16), S1[:],
                                        EXPA, EXPB, op0=OP.mult, op1=OP.add,
                                    )
                                    esc = (e0, e1)
                                else:
                                    e8 = escp.tile([128, 2 * QB], fp8,
                                                   tag="e8", name="e8", bufs=4)
                                    for c in range(2):
                                        nc.scalar.activation(
                                            e8[:, ds(c * QB, QB)],
                                            (S0 if c == 0 else S1)[:],
                                            AF.Exp, scale=0.125,
                                            bias=negc_col[:],
                                        )
                                    esc = e8
                                if blk0:
                                    # deferred K/Q projections: one per iter
                                    if 1 <= j2 <= 8:
                                        kind, m, t = KQ_REST[j2 - 1]
                                        pk = scps.tile([128, QB], fp32,
                                                       tag="sc",
                                                       name=f"p{kind}{m}{t}")
                                        kq_mm(pk, kind, m, t)
                                        kq_evac(pk, kind, m, t)
                                    # V projection: one chunk pair per iter
                                    pv2 = scps.tile([128, QB], fp32,
                                                    tag="sc", name=f"pv{j2}")
                                    for c in range(2):
                                        jv = jj + c
                                        for k2 in range(2):
                                            nc.tensor.matmul(
                                                pv2[:, ds(c * 256, 256)],
                                                xn[k2][:, ts(jv, 128)],
                                                wb[("v", k2)][:],
                                                start=(k2 == 0),
                                                stop=(k2 == 1),
                                            )
                                    for c in range(2):
                                        jv = jj + c
                                        src = pv2[:, ds(c * 256, 256)].rearrange(
                                            "p (h x) -> p h x", h=HEADS
                                        )
                                        if j2 in VT8_SLOT:
                                            nc.vector.tensor_copy(
                                                VT8[:, VT8_SLOT[j2], c, :, 0:D],
                                                src,
                                            )
                                        else:
                                            nc.vector.tensor_copy(
                                                VTb[:, VTB_SLOT[jv], :, 0:D],
                                                src,
                                            )
                                # prev-block drain deferred to j2==1: the
                                # new block's first two score/exp iterations
                                # land ahead of the oa/rz chain, bridging
                                # the PE-activity dip that tripped the HAM
                                # clock gate cold at every block boundary
                                if j2 == 1 and carry is not None:
                                    cavp, ch, cpend = carry
                                    for (pj2, pesc, pfirst) in cpend:
                                        emit_av(cavp, ch, pj2, pesc, pfirst,
                                                pj2 == NP2 - 1)
                                    oa = oap.tile([D + 1, QB], fp32, tag="oa")
                                    nc.vector.tensor_copy(oa[:], cavp[:])
                                    zrow = rzp.tile([1, QB], fp32,
                                                    tag="zrow", bufs=2)
                                    nc.sync.dma_start(zrow[:], oa[D : D + 1, :])
                                    rzf = rzp.tile([1, QB], fp32, tag="rzf",
                                                   bufs=2)
                                    nc.vector.reciprocal_approx_fast(
                                        rzf[:], zrow[:]
                                    )
                                    rzc = rzp.tile([1, QB], bf16, tag="rzc",
                                                   bufs=2)
                                    nc.vector.tensor_copy(rzc[:], rzf[:])
                                    deferred = carry_meta + (oa, rzc)
                                    carry = None
                                if j2 == 4 and deferred is not None:
                                    emit_norm(deferred)
                                    deferred = None
                                if projblk and j2 == 6:
                                    # qb0 output projection: outT cols 0-1023
                                    # final since the previous block's norm
                                    for m in range(2):
                                        ppq = scps.tile([128, QB], fp32,
                                                        tag="sc",
                                                        name=f"ppq0{m}")
                                        for t in range(2):
                                            for k2 in range(2):
                                                nc.tensor.matmul(
                                                    ppq[:, ts(t, 512)],
                                                    wb[("p", k2)][:, ts(m, 128)],
                                                    outT[k2][:, ts(t, 512)],
                                                    start=(k2 == 0),
                                                    stop=(k2 == 1),
                                                )
                                        ysb = ysp.tile([128, QB], fp32,
                                                       tag="ysb")
                                        nc.vector.tensor_tensor(
                                            ysb[:], ppq[:], xb2[m][:, 0:QB],
                                            op=OP.add,
                                        )
                                        for st in range(2):
                                            nc.sync.dma_start(
                                                yo[ts(m, 128),
                                                   ds(st * 512, 512)],
                                                ysb[:, ds(st * 512, 512)],
                                            )
                                if len(pending) >= 2:
                                    (pj2, pesc, pfirst) = pending.pop(0)
                                    emit_av(avp, h, pj2, pesc, pfirst, False)
                                pending.append((j2, esc, j2 == 0))
                            carry = (avp, h, pending)
                            carry_meta = (th, hp, qb)
                        if qb == 0:
                            xb2 = []
                            for m in range(2):
                                x2 = xb2p.tile([128, NLOC], fp32,
                                               tag=f"xb2{m}", name=f"xb2{m}")
                                nc.vector.tensor_scalar_add(
                                    x2[:], xh[(m, 0)][:], bcols[("p2", m)][:]
                                )
                                xb2.append(x2)

                    # drain last block
                    cavp, ch, cpend = carry
                    for (pj2, pesc, pfirst) in cpend:
                        emit_av(cavp, ch, pj2, pesc, pfirst, pj2 == NP2 - 1)
                    oa = oap.tile([D + 1, QB], fp32, tag="oa")
                    nc.vector.tensor_copy(oa[:], cavp[:])
                    zrow = rzp.tile([1, QB], fp32, tag="zrow", bufs=2)
                    nc.sync.dma_start(zrow[:], oa[D : D + 1, :])
                    rzf = rzp.tile([1, QB], fp32, tag="rzf", bufs=2)
                    nc.vector.reciprocal_approx_fast(rzf[:], zrow[:])
                    rzc = rzp.tile([1, QB], bf16, tag="rzc", bufs=2)
                    nc.vector.tensor_copy(rzc[:], rzf[:])
                    emit_norm(carry_meta + (oa, rzc))
                    # qb1 output projection + residual + stores
                    for m in range(2):
                        pp = scps.tile([128, QB], fp32, tag="sc",
                                       name=f"ppq1{m}")
                        for t in (2, 3):
                            for k2 in range(2):
                                nc.tensor.matmul(
                                    pp[:, ts(t - 2, 512)],
                                    wb[("p", k2)][:, ts(m, 128)],
                                    outT[k2][:, ts(t, 512)],
                                    start=(k2 == 0), stop=(k2 == 1),
                                )
                        ysb = ysp.tile([128, QB], fp32, tag="ysb")
                        nc.vector.tensor_tensor(
                            ysb[:], pp[:], xb2[m][:, ds(QB, QB)], op=OP.add
                        )
                        for st in range(2):
                            nc.sync.dma_start(
                                yo[ts(m, 128), ds(QB + st * 512, 512)],
                                ysb[:, ds(st * 512, 512)],
                            )

    nc.compile()
    return nc


def _get_nc():
    if "nc" not in _cached:
        _cached["nc"] = _build_nc()
    return _cached["nc"]


def make_in_maps(x, gn_weight, gn_bias, wq, bq, wk, bk, wv, bv, wp, bp):
    f32 = np.float32
    x = np.asarray(x, f32).reshape(B, C, NTOK)
    bp_eff = np.asarray(bp, f32) + np.asarray(wp, f32) @ np.asarray(bv, f32)
    base = dict(
        wqT=np.ascontiguousarray(np.asarray(wq, f32).T),
        wkT=np.ascontiguousarray(np.asarray(wk, f32).T),
        wvT=np.ascontiguousarray(np.asarray(wv, f32).T),
        wpT=np.ascontiguousarray(np.asarray(wp, f32).T),
        bqc=np.asarray(bq, f32).reshape(C, 1),
        bkc=np.asarray(bk, f32).reshape(C, 1),
        bpc2=bp_eff.reshape(C, 1),
        gnw=np.asarray(gn_weight, f32).reshape(C, 1),
        gnb=np.asarray(gn_bias, f32).reshape(C, 1),
    )
    in_maps = []
    for core in range(8):
        b, half = core // 2, core % 2
        xbv = x[b]
        if half == 1:
            xbv = np.concatenate([xbv[:, NLOC:], xbv[:, :NLOC]], axis=1)
        m = dict(base)
        m["xb"] = np.ascontiguousarray(xbv)
        in_maps.append(m)
    return in_maps


def assemble_output(results):
    out = np.empty((B, C, NTOK), np.float32)
    for core in range(8):
        b, half = core // 2, core % 2
        out[b][:, half * NLOC : (half + 1) * NLOC] = results[core]["y"]
    return out.reshape(B, C, H, W)


def kernel(x, gn_weight, gn_bias, wq, bq, wk, bk, wv, bv, wp, bp):
    from concourse.bass_utils import run_bass_kernel_spmd

    nc = _get_nc()
    in_maps = make_in_maps(
        x, gn_weight, gn_bias, wq, bq, wk, bk, wv, bv, wp, bp
    )
    res = run_bass_kernel_spmd(nc, in_maps, list(range(8)))
    return assemble_output(res.results)
